# revision 19
# baseline (speedup 1.0000x reference)
"""Trainium2 Bass kernel for nn_Concatenation_90701119357422.

Computes, for full inputs:
    ret  = mean(ret_feat, axis=1) @ Wp.T + bp          # [B, H]
    out  = concat([h, ret[batch]], -1) @ Wl.T + bl     # [N, H]

Strategy (8 cores, data-parallel over N), final:
  - Fold the per-row gather into h on the host:  with W1 = Wl[:, :H] and
    ret2 = ret @ Wl[:, H:].T + bl,  solve  W1 M = ret2.T  (256x256, fp64)
    and set  h' = h + M.T[batch].  Then  out = h' @ W1.T  exactly — the
    device runs a PURE GEMM, no gather, no one-hot.
  - Transposed formulation on device: out_t[o, r] = sum_f W1[o, f] h'[f, r].
    The four 128x128 W1 tiles are PE-stationary; h' streams as the moving
    operand 512 rows per matmul (weight-outer sweeps amortize LDWEIGHTS).
  - fp16 h' in, fp16 out, feature-major layouts on both sides so every DMA
    descriptor is 4 KB contiguous per partition; host transposes back.
"""

import os
import sys

import numpy as np

for _p in ("/opt/trn_rl_repo", "/root/.axon_site/_ro/trn_rl_repo"):
    if os.path.isdir(_p) and _p not in sys.path:
        sys.path.append(_p)

import concourse.bass as bass
import concourse.mybir as mybir
import concourse.tile as tile
from concourse import bacc
from concourse.bass_utils import run_bass_kernel_spmd

N_TOTAL = 262144
B = 64
K = 16
H = 256
R = 512
N_CORES = 8
SHARD = N_TOTAL // N_CORES  # 32768

CHUNK = 2048                 # rows per pipeline chunk (4 KB DMA descriptors)
BLK = 512                    # rows per matmul block (one PSUM bank)
F32 = mybir.dt.float32
F16 = mybir.dt.float16


def build_program(shard_rows: int = SHARD):
    assert shard_rows % CHUNK == 0
    n_chunks = shard_rows // CHUNK
    n_blocks = CHUNK // BLK  # 4

    nc = bacc.Bacc("TRN2", target_bir_lowering=False, debug=False)

    # feature-major fp16 h' halves, interleaved per partition for DRAM
    # locality: hab[f, 0, r] = h'[r, f], hab[f, 1, r] = h'[r, 128+f]
    hab_d = nc.dram_tensor("hab", [128, 2, shard_rows], F16, kind="ExternalInput").ap()
    # 4 stationary tiles: W_fA_oA, W_fB_oA, W_fA_oB, W_fB_oB
    wstk_d = nc.dram_tensor("wstk", [128, 4, 128], F16, kind="ExternalInput").ap()
    # feature-major fp16 output halves
    outa_d = nc.dram_tensor("outa", [128, shard_rows], F16, kind="ExternalOutput").ap()
    outb_d = nc.dram_tensor("outb", [128, shard_rows], F16, kind="ExternalOutput").ap()

    with tile.TileContext(nc) as tc:
        with (
            tc.tile_pool(name="const", bufs=1) as cpool,
            tc.tile_pool(name="psum", bufs=1, space="PSUM") as ppool,
            tc.tile_pool(name="hin", bufs=6) as hpool,
            tc.tile_pool(name="outp", bufs=4) as opool,
        ):
            wsb = cpool.tile([128, 4, 128], F16)
            nc.scalar.dma_start(wsb[:], wstk_d[:])

            for ci in range(n_chunks):
                r0 = ci * CHUNK
                hab = hpool.tile([128, 2, CHUNK], F16, tag="hab")
                nc.sync.dma_start(out=hab[:], in_=hab_d[:, :, r0 : r0 + CHUNK])
                ha = hab[:, 0]
                hb = hab[:, 1]

                outA = opool.tile([128, CHUNK], F16, tag="oA")
                outB = opool.tile([128, CHUNK], F16, tag="oB")
                sls = [slice(BLK * j, BLK * (j + 1)) for j in range(n_blocks)]
                pAs = [ppool.tile([128, BLK], F32, tag="pA", bufs=n_blocks, name=f"pA{j}")
                       for j in range(n_blocks)]
                pBs = [ppool.tile([128, BLK], F32, tag="pB", bufs=n_blocks, name=f"pB{j}")
                       for j in range(n_blocks)]
                # weight-outer sweeps: each stationary tile loaded once per chunk
                for j in range(n_blocks):
                    nc.tensor.matmul(pAs[j][:], wsb[:, 0], ha[:, sls[j]], start=True, stop=False)
                for j in range(n_blocks):
                    nc.tensor.matmul(pAs[j][:], wsb[:, 1], hb[:, sls[j]], start=False, stop=True)
                for j in range(n_blocks):
                    nc.scalar.copy(outA[:, sls[j]], pAs[j][:])
                nc.gpsimd.dma_start(out=outa_d[:, r0 : r0 + CHUNK], in_=outA[:])
                for j in range(n_blocks):
                    nc.tensor.matmul(pBs[j][:], wsb[:, 2], ha[:, sls[j]], start=True, stop=False)
                for j in range(n_blocks):
                    nc.tensor.matmul(pBs[j][:], wsb[:, 3], hb[:, sls[j]], start=False, stop=True)
                if ci < n_chunks - 1:
                    for j in range(n_blocks):
                        nc.vector.tensor_copy(outB[:, sls[j]], pBs[j][:])
                    nc.gpsimd.dma_start(out=outb_d[:, r0 : r0 + CHUNK], in_=outB[:])
                else:
                    # drain the tail faster: per-block output DMA on the
                    # last chunk so writes overlap the remaining copies
                    for j in range(n_blocks):
                        nc.vector.tensor_copy(outB[:, sls[j]], pBs[j][:])
                        nc.gpsimd.dma_start(
                            out=outb_d[:, r0 + BLK * j : r0 + BLK * (j + 1)],
                            in_=outB[:, sls[j]],
                        )

    nc.compile()
    return nc


def prep_inputs(h, ret_feat, batch, Wp, bp, Wl, bl, shard_rows: int = SHARD,
                n_cores: int = N_CORES):
    """Host-side prep: fold gather into h', shard + cast + pre-transpose."""
    h = np.asarray(h, dtype=np.float32)
    Wl = np.asarray(Wl, dtype=np.float32)
    Wp = np.asarray(Wp, dtype=np.float32)
    bp = np.asarray(bp, dtype=np.float32)
    bl = np.asarray(bl, dtype=np.float32)
    ret_feat = np.asarray(ret_feat, dtype=np.float32)
    batch = np.asarray(batch)

    # pooled ret table: ret2 = ((mean_k rf) @ Wp.T + bp) @ Wl[:,H:].T + bl
    W1 = Wl[:, :H].astype(np.float64)            # [H, H]
    ret = ret_feat.astype(np.float64).mean(axis=1) @ Wp.astype(np.float64).T + bp
    ret2 = ret @ Wl[:, H:].astype(np.float64).T + bl   # [B, H]
    M = np.linalg.solve(W1, ret2.T)              # [H, B]:  W1 @ M = ret2.T

    # h' = h + M.T[batch]  (so that h' @ W1.T = h @ W1.T + ret2[batch])
    hp16 = (h + M.T.astype(np.float32)[batch]).astype(np.float16)

    W1_16 = Wl[:, :H].astype(np.float16)
    wstk = np.empty((128, 4, 128), dtype=np.float16)
    wstk[:, 0, :] = W1_16[:128, :128].T
    wstk[:, 1, :] = W1_16[:128, 128:256].T
    wstk[:, 2, :] = W1_16[128:256, :128].T
    wstk[:, 3, :] = W1_16[128:256, 128:256].T

    in_maps = []
    for i in range(n_cores):
        s = slice(i * shard_rows, (i + 1) * shard_rows)
        hs = hp16[s]
        hab = np.empty((128, 2, shard_rows), dtype=np.float16)
        hab[:, 0] = hs[:, :128].T
        hab[:, 1] = hs[:, 128:].T
        in_maps.append({"hab": hab, "wstk": wstk})
    return in_maps


_PROGRAM_CACHE = {}


def _get_program(shard_rows: int = SHARD):
    if shard_rows not in _PROGRAM_CACHE:
        _PROGRAM_CACHE[shard_rows] = build_program(shard_rows)
    return _PROGRAM_CACHE[shard_rows]


def kernel(h, ret_feat, batch, Wp, bp, Wl, bl):
    nc = _get_program(SHARD)
    in_maps = prep_inputs(h, ret_feat, batch, Wp, bp, Wl, bl)
    res = run_bass_kernel_spmd(nc, in_maps, list(range(N_CORES)))
    out = np.empty((N_TOTAL, H), dtype=np.float32)
    for i in range(N_CORES):
        s = slice(i * SHARD, (i + 1) * SHARD)
        out[s, :128] = res.results[i]["outa"].T
        out[s, 128:] = res.results[i]["outb"].T
    return out
